# revision 8
# baseline (speedup 1.0000x reference)
"""Trainium2 Bass kernel for cosine-similarity contrastive loss (CosSimLoss).

reference:
    p = l2norm(pred).reshape(-1, C); t = l2norm(target).reshape(-1, C)
    logits = (p @ t.T) * e^0.5
    loss = mean(logsumexp(logits, axis=1) - diag(logits))

Strategy (8 NeuronCores, data parallel over the N = B*L = 8192 row dim).
Each core gets a 1024-row pred shard plus the full target and computes its
1024 x 8192 logits tile. Key points:

- fp8 (e4m3) DoubleRow matmuls: both operands cast to fp8 and packed as
  (c, c+256) byte pairs inside u16 words; one matmul consumes 256
  contraction rows (2 fp8 MACs/cell/cycle) - 2x bf16 TensorE throughput.
  Contraction pair h covers c in {128h+k', 128h+k'+256} for k' in [0,128).
- pred is NOT normalized before the matmul: its norm reciprocal rp, the
  e^0.5 temperature, and the fp8 x16 target scale all fold into the EXP
  activation's per-partition scale vector. Target rows are normalized (x16)
  during the fp32->fp8 cast (a per-column scale can't be applied after the
  matmul).
- transposes (contraction dim -> partitions) are SBUF->SBUF DMA xbar
  transposes of packed u16 tiles - no DRAM bounce, no TensorE transposes,
  so all 8 PSUM banks double-buffer the matmul->exp pipeline.
- loads use 8KB/partition descriptors ([128, 8, 512] "octo" tiles, row =
  r0 + 8p + q). Row order is scrambled across partitions, which is harmless:
  the loss is a mean over rows of (lse - diag) computed in one consistent
  scrambled order, and the lse sum over columns is order-invariant.
- row norms via one-pass DVE bn_stats (sum x^2 from even/odd mean+var).
- the diagonal is computed exactly in fp32 from the pred shard and the
  matching target rows (separate pre-sliced "td" input, SPMD-friendly),
  overlapped with the matmul phase. |cos|<=1 so exp never overflows.

Per-row (logsumexp - diag) partials return as [128, 8] per core; the host
sums and divides by N.
"""
import math

import numpy as np

import concourse.bacc as bacc
import concourse.mybir as mybir
import concourse.tile as tile
from concourse.bass_utils import run_bass_kernel_spmd

F32 = mybir.dt.float32
BF16 = mybir.dt.bfloat16
FP8 = mybir.dt.float8e4
U16 = mybir.dt.uint16
AF = mybir.ActivationFunctionType
ALU = mybir.AluOpType
AXIS = mybir.AxisListType
PM = mybir.MatmulPerfMode

TEMPERATURE = 0.5
SCALE = float(math.exp(TEMPERATURE))
TSCALE = 16.0  # fp8 scale applied to normalized target rows

# Full problem config (hardcoded per contest rules).
B, L, C = 4, 2048, 512
N_CORES = 8
N_TOTAL = B * L                  # 8192
M_LOCAL = N_TOTAL // N_CORES     # 1024 rows per core

NB = 4                           # n blocks of 2048 columns (psum tiles)
BLK = N_TOTAL // NB              # 2048
QR = 1024                        # rows per octo load tile [128, 8, 512]
HALF = C // 2                    # 256: fp8 pair partner offset
MT = M_LOCAL // 128              # 8 output row tiles
JT = BLK // 512                  # 4 psum 512-col slices per block
OPB = BLK // QR                  # 2 octo loads per n-block


def octo_dram_ap(t2d, r0, rows=QR):
    # DRAM rows [r0, r0+rows) as [128 part, rows//128, c]; row = r0 + 8p + q
    return t2d[r0:r0 + rows, :].rearrange("(p q) c -> p q c", p=128)


def build_nc():
    m_local, n, c = M_LOCAL, N_TOTAL, C

    nc = bacc.Bacc("TRN2", target_bir_lowering=False, debug=False)
    pred = nc.dram_tensor("pred", [m_local, c], F32, kind="ExternalInput").ap()
    tgt = nc.dram_tensor("tgt", [n, c], F32, kind="ExternalInput").ap()
    td = nc.dram_tensor("td", [m_local, c], F32, kind="ExternalInput").ap()
    out = nc.dram_tensor("out", [128, MT], F32, kind="ExternalOutput").ap()

    with tile.TileContext(nc) as tc:
        with (
            tc.tile_pool(name="pq", bufs=1) as p_pool,        # pred + td octos
            tc.tile_pool(name="tq", bufs=6) as t_pool,        # target octos
            tc.tile_pool(name="pk", bufs=10) as pk_pool,      # packed fp8 chunks
            tc.tile_pool(name="pT", bufs=1) as pT_pool,       # transposed packs
            tc.tile_pool(name="st", bufs=1) as stats_pool,
            tc.tile_pool(name="sc", bufs=4) as scratch_pool,
            tc.tile_pool(name="esc", bufs=2) as esc_pool,
            tc.tile_pool(name="psum", bufs=2, space="PSUM") as psum_pool,
        ):
            # ---------- persistent stats ----------
            bnp = stats_pool.tile([128, MT * 6], F32)     # pred bn stats
            bnd = stats_pool.tile([128, MT * 6], F32)     # td bn stats
            bnt = stats_pool.tile([128, 64 * 6], F32)     # tgt bn stats
            rp16 = stats_pool.tile([128, MT], F32)        # SCALE/TSCALE/||p||
            rp = stats_pool.tile([128, MT], F32)
            rtd = stats_pool.tile([128, MT], F32)
            rtt = stats_pool.tile([128, 64], F32)         # TSCALE/||t||
            d0 = stats_pool.tile([128, MT], F32)          # sum pred*td
            sume = stats_pool.tile([128, MT * NB], F32)   # exp row sums

            def bn_rsqrt(dst_ap, bn, k0, ncols, scale):
                """dst_ap[:, :ncols] = scale/sqrt(sum x^2) from bn stats cols
                [k0, k0+ncols). sum x^2 = cnt*(var_e + mean_e^2 + var_o +
                mean_o^2) with cnt=256 for each half (bn output stores
                cnt*var directly)."""
                v = bn[:, k0 * 6:(k0 + ncols) * 6].rearrange(
                    "p (k s) -> p k s", s=6)
                me, ve = v[:, :, 1], v[:, :, 2]
                mo, vo = v[:, :, 4], v[:, :, 5]
                t1 = scratch_pool.tile([128, ncols], F32, name="bn1")
                t2 = scratch_pool.tile([128, ncols], F32, name="bn2")
                nc.vector.scalar_tensor_tensor(
                    t1[:], me, 256.0, me, ALU.mult, ALU.mult)
                nc.vector.scalar_tensor_tensor(
                    t2[:], mo, 256.0, mo, ALU.mult, ALU.mult)
                nc.vector.tensor_add(t1[:], t1[:], ve)
                nc.vector.tensor_add(t2[:], t2[:], vo)
                nc.vector.tensor_add(t1[:], t1[:], t2[:])
                # scale * exp(-0.5 ln(sumsq)); ln/exp share one ACT table set
                nc.scalar.activation(t2[:], t1[:], AF.Ln)
                nc.scalar.activation(dst_ap, t2[:], AF.Exp, scale=-0.5)
                if scale != 1.0:
                    nc.vector.tensor_scalar_mul(dst_ap, dst_ap, scale)

            def pack_pairs(dst_pk, src_ap, engine, rt_ap=None):
                """fp32 [128, 512] -> fp8 pair-packed [128, 512B]: byte 2k+i
                = x[:, 256i + k] (optionally * rt row scale)."""
                pkv = dst_pk[:].rearrange("p (k two) -> p two k", two=2)
                sv = src_ap.rearrange("p (two k) -> p two k", two=2)
                if rt_ap is None:
                    engine.tensor_copy(pkv, sv)
                else:
                    engine.tensor_scalar_mul(pkv, sv, rt_ap)

            # ---------- pred + td loads ----------
            pq = p_pool.tile([128, MT, c], F32)            # kept until tail
            nc.sync.dma_start(pq[:], octo_dram_ap(pred, 0))
            tdq = p_pool.tile([128, MT, c], F32)           # kept until tail
            nc.sync.dma_start(tdq[:], octo_dram_ap(td, 0))

            # pred stats -> rp, rp16
            for s in range(MT):
                nc.vector.bn_stats(bnp[:, s * 6:(s + 1) * 6], pq[:, s])
            bn_rsqrt(rp[:], bnp, 0, MT, 1.0)
            nc.vector.tensor_scalar_mul(rp16[:], rp[:], SCALE / TSCALE)

            # pred raw fp32 -> bf16 -> xbar transpose (middle dim = c-chunk)
            # -> fp8 cast into the contiguous [128, 2, M] dual-fp8 weights
            # layout walrus requires (k-pair dim NOT byte-interleaved).
            pbf = p_pool.tile([128, MT, c], BF16, name="pbf")
            pTb = pT_pool.tile([128, 4, m_local], BF16, name="pTb")
            for s in range(MT):
                nc.vector.tensor_copy(pbf[:, s], pq[:, s])
                # out[p, e, a] = in[a, 128e+p]: e = c-chunk, a = m col
                nc.sync.dma_start_transpose(
                    pTb[:, :, s * 128:(s + 1) * 128], pbf[:, s])
            # ppT[:, h, i, m] = pred[m-col, 128h + 256i + k'] in fp8
            ppT = pT_pool.tile([128, 2, 2, m_local], FP8, name="ppT")
            for h in range(2):
                for i in range(2):
                    nc.vector.tensor_copy(ppT[:, h, i], pTb[:, h + 2 * i])

            # ---------- target pipeline ----------
            tT = [pT_pool.tile([128, 2, 2 * BLK], FP8, name=f"tT{g}")
                  for g in range(NB)]

            def emit_tgt_loads(g):
                tqs = []
                for o in range(OPB):
                    tq = t_pool.tile([128, 8, c], F32, name="tq")
                    nc.sync.dma_start(
                        tq[:], octo_dram_ap(tgt, (g * OPB + o) * QR))
                    tqs.append(tq)
                return tqs

            def emit_tgt_prep(g, tqs):
                dstu = tT[g][:].bitcast(U16)               # [128, 2, BLK]
                for o in range(OPB):
                    tq = tqs[o]
                    cg0 = (g * OPB + o) * 8
                    for q in range(8):
                        nc.vector.bn_stats(
                            bnt[:, (cg0 + q) * 6:(cg0 + q + 1) * 6], tq[:, q])
                    bn_rsqrt(rtt[:, cg0:cg0 + 8], bnt, cg0, 8, TSCALE)
                    for q in range(8):
                        ci = cg0 + q
                        bl = o * 8 + q                     # chunk idx in block
                        pk = pk_pool.tile([128, 2 * HALF], FP8, name="tpk")
                        pack_pairs(pk, tq[:, q], nc.gpsimd,
                                   rtt[:, ci:ci + 1])
                        nc.sync.dma_start_transpose(
                            dstu[:, :, bl * 128:(bl + 1) * 128],
                            pk[:].bitcast(U16))

            def emit_mm_block(g):
                tv = [tT[g][:, h].rearrange("p (n two) -> p two n", two=2)
                      for h in range(2)]
                for m in range(MT):
                    ps = psum_pool.tile([128, BLK], F32, name="ps")
                    for h in range(2):
                        lhs = ppT[:, h, :, m * 128:(m + 1) * 128]
                        for j in range(JT):
                            nc.tensor.matmul(
                                ps[:, j * 512:(j + 1) * 512],
                                lhs,
                                tv[h][:, :, j * 512:(j + 1) * 512],
                                start=(h == 0), stop=(h == 1),
                                perf_mode=PM.DoubleRow)
                    esc = esc_pool.tile([128, BLK], BF16, name="esc")
                    nc.scalar.activation(
                        esc[:], ps[:], AF.Exp, scale=rp16[:, m:m + 1],
                        accum_out=sume[:, m * NB + g:m * NB + g + 1])

            # one-block lookahead: target prep for g+1 overlaps matmuls of g;
            # loads for g+1 are issued on Sync before g's transposes so the
            # DMA stream keeps ahead of the compute pipeline.
            tq0 = emit_tgt_loads(0)
            tq1 = emit_tgt_loads(1)
            emit_tgt_prep(0, tq0)
            tq2 = emit_tgt_loads(2)
            emit_tgt_prep(1, tq1)
            emit_mm_block(0)
            tq3 = emit_tgt_loads(3)
            emit_tgt_prep(2, tq2)
            emit_mm_block(1)
            emit_tgt_prep(3, tq3)
            emit_mm_block(2)
            emit_mm_block(3)

            # ---------- tail: exact diag + lse - diag ----------
            for s in range(MT):
                nc.vector.bn_stats(bnd[:, s * 6:(s + 1) * 6], tdq[:, s])
                sq = scratch_pool.tile([128, c], F32, name="sqd")
                nc.vector.scalar_tensor_tensor(
                    sq[:], pq[:, s], 1.0, tdq[:, s], ALU.mult, ALU.mult,
                    accum_out=d0[:, s:s + 1])
            bn_rsqrt(rtd[:], bnd, 0, MT, 1.0)
            dtmp = scratch_pool.tile([128, MT], F32)
            nc.vector.tensor_mul(dtmp[:], d0[:], rp[:])
            diag = scratch_pool.tile([128, MT], F32)
            nc.vector.scalar_tensor_tensor(
                diag[:], dtmp[:], SCALE, rtd[:], ALU.mult, ALU.mult)

            rowsum = scratch_pool.tile([128, MT], F32)
            nc.vector.tensor_reduce(
                rowsum[:], sume[:].rearrange("p (m g) -> p m g", g=NB),
                axis=AXIS.X, op=ALU.add)
            lse = scratch_pool.tile([128, MT], F32)
            nc.scalar.activation(lse[:], rowsum[:], AF.Ln)
            losst = scratch_pool.tile([128, MT], F32)
            nc.vector.tensor_sub(losst[:], lse[:], diag[:])
            nc.sync.dma_start(out[:], losst[:])

    nc.compile()
    return nc


_NC_CACHE = {}


def _get_nc():
    key = (M_LOCAL, N_TOTAL, C)
    if key not in _NC_CACHE:
        _NC_CACHE[key] = build_nc()
    return _NC_CACHE[key]


def run_cores(pred2d, tgt2d, trace=False):
    """Run the SPMD program on cores 0..7; returns (partials [8,128,mt], res)."""
    nc = _get_nc()
    in_maps = []
    for ci in range(N_CORES):
        r0 = ci * M_LOCAL
        in_maps.append({
            "pred": np.ascontiguousarray(pred2d[r0:r0 + M_LOCAL]),
            "tgt": np.ascontiguousarray(tgt2d),
            "td": np.ascontiguousarray(tgt2d[r0:r0 + M_LOCAL]),
        })
    res = run_bass_kernel_spmd(nc, in_maps, list(range(N_CORES)), trace=trace)
    partials = np.stack([res.results[i]["out"] for i in range(N_CORES)])
    return partials, res


def kernel(pred, target):
    pred2d = np.asarray(pred, dtype=np.float32).reshape(-1, C)
    tgt2d = np.asarray(target, dtype=np.float32).reshape(-1, C)
    partials, _ = run_cores(pred2d, tgt2d)
    loss = partials.astype(np.float64).sum() / float(N_TOTAL)
    return np.float32(loss)


# revision 15
# speedup vs baseline: 1.0085x; 1.0085x over previous
"""Trainium2 Bass kernel for cosine-similarity contrastive loss (CosSimLoss).

reference:
    p = l2norm(pred).reshape(-1, C); t = l2norm(target).reshape(-1, C)
    logits = (p @ t.T) * e^0.5
    loss = mean(logsumexp(logits, axis=1) - diag(logits))

Strategy (8 NeuronCores, data parallel over the N = B*L = 8192 row dim).
Each core gets a 1024-row pred shard plus the full target and computes its
1024 x 8192 logits tile. Key points:

- fp8 (e4m3) DoubleRow matmuls: both operands cast to fp8 and packed as
  (c, c+256) byte pairs inside u16 words; one matmul consumes 256
  contraction rows (2 fp8 MACs/cell/cycle) - 2x bf16 TensorE throughput.
  Contraction pair h covers c in {128h+k', 128h+k'+256} for k' in [0,128).
- pred is NOT normalized before the matmul: its norm reciprocal rp, the
  e^0.5 temperature, and the fp8 x16 target scale all fold into the EXP
  activation's per-partition scale vector. Target rows are normalized (x16)
  during the fp32->fp8 cast (a per-column scale can't be applied after the
  matmul).
- transposes (contraction dim -> partitions) are SBUF->SBUF DMA xbar
  transposes of packed u16 tiles - no DRAM bounce, no TensorE transposes,
  so all 8 PSUM banks double-buffer the matmul->exp pipeline.
- loads use 8KB/partition descriptors ([128, 8, 512] "octo" tiles, row =
  r0 + 8p + q). Row order is scrambled across partitions, which is harmless:
  the loss is a mean over rows of (lse - diag) computed in one consistent
  scrambled order, and the lse sum over columns is order-invariant.
- row norms via one-pass DVE bn_stats (sum x^2 from even/odd mean+var).
- the diagonal is computed exactly in fp32 from the pred shard and the
  matching target rows (separate pre-sliced "td" input, SPMD-friendly),
  overlapped with the matmul phase. |cos|<=1 so exp never overflows.

Per-row (logsumexp - diag) partials return as [128, 8] per core; the host
sums and divides by N.
"""
import math

import numpy as np

import concourse.bacc as bacc
import concourse.mybir as mybir
import concourse.tile as tile
from concourse.bass_utils import run_bass_kernel_spmd

F32 = mybir.dt.float32
BF16 = mybir.dt.bfloat16
FP8 = mybir.dt.float8e4
U16 = mybir.dt.uint16
AF = mybir.ActivationFunctionType
ALU = mybir.AluOpType
AXIS = mybir.AxisListType
PM = mybir.MatmulPerfMode

TEMPERATURE = 0.5
SCALE = float(math.exp(TEMPERATURE))
TSCALE = 16.0  # fp8 scale applied to normalized target rows

# Full problem config (hardcoded per contest rules).
B, L, C = 4, 2048, 512
N_CORES = 8
N_TOTAL = B * L                  # 8192
M_LOCAL = N_TOTAL // N_CORES     # 1024 rows per core

NB = 4                           # n blocks of 2048 columns (psum tiles)
BLK = N_TOTAL // NB              # 2048
QR = 1024                        # rows per octo load tile [128, 8, 512]
HALF = C // 2                    # 256: fp8 pair partner offset
MT = M_LOCAL // 128              # 8 output row tiles
JT = BLK // 512                  # 4 psum 512-col slices per block
OPB = BLK // QR                  # 2 octo loads per n-block


def octo_dram_ap(t2d, r0, rows=QR):
    # DRAM rows [r0, r0+rows) as [128 part, rows//128, c]; row = r0 + 8p + q
    return t2d[r0:r0 + rows, :].rearrange("(p q) c -> p q c", p=128)


def build_nc():
    m_local, n, c = M_LOCAL, N_TOTAL, C

    nc = bacc.Bacc("TRN2", target_bir_lowering=False, debug=False)
    pred = nc.dram_tensor("pred", [m_local, c], F32, kind="ExternalInput").ap()
    tgt = nc.dram_tensor("tgt", [n, c], F32, kind="ExternalInput").ap()
    td = nc.dram_tensor("td", [m_local, c], F32, kind="ExternalInput").ap()
    out = nc.dram_tensor("out", [128, MT], F32, kind="ExternalOutput").ap()

    with tile.TileContext(nc) as tc:
        with (
            tc.tile_pool(name="pq", bufs=1) as p_pool,        # pred + td octos
            tc.tile_pool(name="tq", bufs=6) as t_pool,        # target octos
            tc.tile_pool(name="pk", bufs=10) as pk_pool,      # packed fp8 chunks
            tc.tile_pool(name="pT", bufs=1) as pT_pool,       # transposed packs
            tc.tile_pool(name="st", bufs=1) as stats_pool,
            tc.tile_pool(name="sc", bufs=4) as scratch_pool,
            tc.tile_pool(name="esc", bufs=2) as esc_pool,
            tc.tile_pool(name="psum", bufs=2, space="PSUM") as psum_pool,
        ):
            # ---------- persistent stats ----------
            bnp = stats_pool.tile([128, MT * 6], F32)     # pred bn stats
            bnd = stats_pool.tile([128, MT * 6], F32)     # td bn stats
            bnt = stats_pool.tile([128, 64 * 6], F32)     # tgt bn stats
            rp16 = stats_pool.tile([128, MT], F32)        # SCALE/TSCALE/||p||
            rp = stats_pool.tile([128, MT], F32)
            rtd = stats_pool.tile([128, MT], F32)
            rtt = stats_pool.tile([128, 64], F32)         # TSCALE/||t||
            d0 = stats_pool.tile([128, MT], F32)          # sum pred*td
            sume = stats_pool.tile([128, MT * NB], F32)   # exp row sums

            def bn_rsqrt(dst_ap, bn, k0, ncols, scale):
                """dst_ap[:, :ncols] = scale/sqrt(sum x^2) from bn stats cols
                [k0, k0+ncols). sum x^2 = cnt*(var_e + mean_e^2 + var_o +
                mean_o^2) with cnt=256 for each half (bn output stores
                cnt*var directly)."""
                v = bn[:, k0 * 6:(k0 + ncols) * 6].rearrange(
                    "p (k s) -> p k s", s=6)
                me, ve = v[:, :, 1], v[:, :, 2]
                mo, vo = v[:, :, 4], v[:, :, 5]
                t1 = scratch_pool.tile([128, ncols], F32, name="bn1")
                t2 = scratch_pool.tile([128, ncols], F32, name="bn2")
                nc.vector.scalar_tensor_tensor(
                    t1[:], me, 256.0, me, ALU.mult, ALU.mult)
                nc.vector.scalar_tensor_tensor(
                    t2[:], mo, 256.0, mo, ALU.mult, ALU.mult)
                nc.vector.tensor_add(t1[:], t1[:], ve)
                nc.vector.tensor_add(t2[:], t2[:], vo)
                nc.vector.tensor_add(t1[:], t1[:], t2[:])
                # scale * exp(-0.5 ln(sumsq)); ln/exp share one ACT table set
                nc.scalar.activation(t2[:], t1[:], AF.Ln)
                nc.scalar.activation(dst_ap, t2[:], AF.Exp, scale=-0.5)
                if scale != 1.0:
                    nc.vector.tensor_scalar_mul(dst_ap, dst_ap, scale)

            def pack_pairs(dst_pk, src_ap, engine, rt_ap=None):
                """fp32 [128, 512] -> fp8 pair-packed [128, 512B]: byte 2k+i
                = x[:, 256i + k] (optionally * rt row scale). Iteration order
                (k outer, i inner) keeps the byte WRITES contiguous; the
                strided fp32 reads are cheap by comparison."""
                pkv = dst_pk[:].rearrange("p (k two) -> p k two", two=2)
                sv = src_ap.rearrange("p (two k) -> p k two", two=2)
                if rt_ap is None:
                    engine.tensor_copy(pkv, sv)
                else:
                    engine.tensor_scalar_mul(pkv, sv, rt_ap)

            # ---------- pred + td loads ----------
            pq = p_pool.tile([128, MT, c], F32)            # kept until tail
            nc.sync.dma_start(pq[:], octo_dram_ap(pred, 0))
            tdq = p_pool.tile([128, MT, c], F32)           # kept until tail
            nc.sync.dma_start(tdq[:], octo_dram_ap(td, 0))

            # pred stats -> rp, rp16
            for s in range(MT):
                nc.vector.bn_stats(bnp[:, s * 6:(s + 1) * 6], pq[:, s])
            bn_rsqrt(rp[:], bnp, 0, MT, 1.0)
            nc.vector.tensor_scalar_mul(rp16[:], rp[:], SCALE / TSCALE)

            # pred raw fp32 -> bf16 -> xbar transpose (middle dim = c-chunk)
            # -> fp8 cast into the contiguous [128, 2, M] dual-fp8 weights
            # layout walrus requires (k-pair dim NOT byte-interleaved).
            pbf = p_pool.tile([128, MT, c], BF16, name="pbf")
            pTb = pT_pool.tile([128, 4, m_local], BF16, name="pTb")
            for s in range(MT):
                nc.vector.tensor_copy(pbf[:, s], pq[:, s])
                # out[p, e, a] = in[a, 128e+p]: e = c-chunk, a = m col
                nc.sync.dma_start_transpose(
                    pTb[:, :, s * 128:(s + 1) * 128], pbf[:, s])
            # ppT[:, h, i, m] = pred[m-col, 128h + 256i + k'] in fp8
            ppT = pT_pool.tile([128, 2, 2, m_local], FP8, name="ppT")
            for h in range(2):
                for i in range(2):
                    nc.vector.tensor_copy(ppT[:, h, i], pTb[:, h + 2 * i])

            # ---------- target pipeline ----------
            tT = [pT_pool.tile([128, 2, 2 * BLK], FP8, name=f"tT{g}")
                  for g in range(NB)]

            def emit_tgt_loads(g):
                tqs = []
                for o in range(OPB):
                    tq = t_pool.tile([128, 8, c], F32, name="tq")
                    nc.sync.dma_start(
                        tq[:], octo_dram_ap(tgt, (g * OPB + o) * QR))
                    tqs.append(tq)
                return tqs

            def emit_tgt_prep(g, tqs):
                dstu = tT[g][:].bitcast(U16)               # [128, 2, BLK]
                for o in range(OPB):
                    tq = tqs[o]
                    cg0 = (g * OPB + o) * 8
                    for q in range(8):
                        nc.vector.bn_stats(
                            bnt[:, (cg0 + q) * 6:(cg0 + q + 1) * 6], tq[:, q])
                    bn_rsqrt(rtt[:, cg0:cg0 + 8], bnt, cg0, 8, TSCALE)
                    for q in range(8):
                        ci = cg0 + q
                        bl = o * 8 + q                     # chunk idx in block
                        pk = pk_pool.tile([128, 2 * HALF], FP8, name="tpk")
                        pack_pairs(pk, tq[:, q], nc.gpsimd,
                                   rtt[:, ci:ci + 1])
                        nc.sync.dma_start_transpose(
                            dstu[:, :, bl * 128:(bl + 1) * 128],
                            pk[:].bitcast(U16))

            def emit_mm_block(g):
                tv = [tT[g][:, h].rearrange("p (n two) -> p two n", two=2)
                      for h in range(2)]
                for m in range(MT):
                    ps = psum_pool.tile([128, BLK], F32, name="ps")
                    for h in range(2):
                        lhs = ppT[:, h, :, m * 128:(m + 1) * 128]
                        for j in range(JT):
                            nc.tensor.matmul(
                                ps[:, j * 512:(j + 1) * 512],
                                lhs,
                                tv[h][:, :, j * 512:(j + 1) * 512],
                                start=(h == 0), stop=(h == 1),
                                perf_mode=PM.DoubleRow)
                    esc = esc_pool.tile([128, BLK], BF16, name="esc")
                    nc.scalar.activation(
                        esc[:], ps[:], AF.Exp, scale=rp16[:, m:m + 1],
                        accum_out=sume[:, m * NB + g:m * NB + g + 1])

            # one-block lookahead: target prep for g+1 overlaps matmuls of g;
            # loads for g+1 are issued on Sync before g's transposes so the
            # DMA stream keeps ahead of the compute pipeline.
            tq0 = emit_tgt_loads(0)
            tq1 = emit_tgt_loads(1)
            emit_tgt_prep(0, tq0)
            tq2 = emit_tgt_loads(2)
            emit_tgt_prep(1, tq1)
            emit_mm_block(0)
            tq3 = emit_tgt_loads(3)
            emit_tgt_prep(2, tq2)
            emit_mm_block(1)
            emit_tgt_prep(3, tq3)
            emit_mm_block(2)
            emit_mm_block(3)

            # ---------- tail: exact diag + lse - diag ----------
            for s in range(MT):
                nc.vector.bn_stats(bnd[:, s * 6:(s + 1) * 6], tdq[:, s])
                sq = scratch_pool.tile([128, c], F32, name="sqd")
                nc.vector.scalar_tensor_tensor(
                    sq[:], pq[:, s], 1.0, tdq[:, s], ALU.mult, ALU.mult,
                    accum_out=d0[:, s:s + 1])
            bn_rsqrt(rtd[:], bnd, 0, MT, 1.0)
            dtmp = scratch_pool.tile([128, MT], F32)
            nc.vector.tensor_mul(dtmp[:], d0[:], rp[:])
            diag = scratch_pool.tile([128, MT], F32)
            nc.vector.scalar_tensor_tensor(
                diag[:], dtmp[:], SCALE, rtd[:], ALU.mult, ALU.mult)

            rowsum = scratch_pool.tile([128, MT], F32)
            nc.vector.tensor_reduce(
                rowsum[:], sume[:].rearrange("p (m g) -> p m g", g=NB),
                axis=AXIS.X, op=ALU.add)
            lse = scratch_pool.tile([128, MT], F32)
            nc.scalar.activation(lse[:], rowsum[:], AF.Ln)
            losst = scratch_pool.tile([128, MT], F32)
            nc.vector.tensor_sub(losst[:], lse[:], diag[:])
            nc.sync.dma_start(out[:], losst[:])

    nc.compile()
    return nc


_NC_CACHE = {}


def _get_nc():
    key = (M_LOCAL, N_TOTAL, C)
    if key not in _NC_CACHE:
        _NC_CACHE[key] = build_nc()
    return _NC_CACHE[key]


def run_cores(pred2d, tgt2d, trace=False):
    """Run the SPMD program on cores 0..7; returns (partials [8,128,mt], res)."""
    nc = _get_nc()
    in_maps = []
    for ci in range(N_CORES):
        r0 = ci * M_LOCAL
        in_maps.append({
            "pred": np.ascontiguousarray(pred2d[r0:r0 + M_LOCAL]),
            "tgt": np.ascontiguousarray(tgt2d),
            "td": np.ascontiguousarray(tgt2d[r0:r0 + M_LOCAL]),
        })
    res = run_bass_kernel_spmd(nc, in_maps, list(range(N_CORES)), trace=trace)
    partials = np.stack([res.results[i]["out"] for i in range(N_CORES)])
    return partials, res


def kernel(pred, target):
    pred2d = np.asarray(pred, dtype=np.float32).reshape(-1, C)
    tgt2d = np.asarray(target, dtype=np.float32).reshape(-1, C)
    partials, _ = run_cores(pred2d, tgt2d)
    loss = partials.astype(np.float64).sum() / float(N_TOTAL)
    return np.float32(loss)


# revision 20
# speedup vs baseline: 2.6094x; 2.5874x over previous
"""Trainium2 Bass kernel for cosine-similarity contrastive loss (CosSimLoss).

reference:
    p = l2norm(pred).reshape(-1, C); t = l2norm(target).reshape(-1, C)
    logits = (p @ t.T) * e^0.5
    loss = mean(logsumexp(logits, axis=1) - diag(logits))

Strategy (8 NeuronCores, data parallel over the N = B*L = 8192 row dim).
Each core gets a 1024-row pred shard plus the full target and computes its
1024 x 8192 logits tile. Key points:

- fp8 (e4m3) DoubleRow matmuls: both operands cast to fp8 and packed as
  (c, c+256) byte pairs inside u16 words; one matmul consumes 256
  contraction rows (2 fp8 MACs/cell/cycle) - 2x bf16 TensorE throughput.
  Contraction pair h covers c in {128h+k', 128h+k'+256} for k' in [0,128).
- pred is NOT normalized before the matmul: its norm reciprocal rp, the
  e^0.5 temperature, and the fp8 x16 target scale all fold into the EXP
  activation's per-partition scale vector. Target rows are normalized (x16)
  during the fp32->fp8 cast (a per-column scale can't be applied after the
  matmul).
- transposes (contraction dim -> partitions) are SBUF->SBUF DMA xbar
  transposes of packed u16 tiles - no DRAM bounce, no TensorE transposes,
  so all 8 PSUM banks double-buffer the matmul->exp pipeline.
- loads use 8KB/partition descriptors ([128, 8, 512] "octo" tiles, row =
  r0 + 8p + q). Row order is scrambled across partitions, which is harmless:
  the loss is a mean over rows of (lse - diag) computed in one consistent
  scrambled order, and the lse sum over columns is order-invariant.
- row norms via one-pass DVE bn_stats (sum x^2 from even/odd mean+var).
- the diagonal is computed exactly in fp32 from the pred shard and the
  matching target rows (separate pre-sliced "td" input, SPMD-friendly),
  overlapped with the matmul phase. |cos|<=1 so exp never overflows.

Per-row (logsumexp - diag) partials return as [128, 8] per core; the host
sums and divides by N.
"""
import math

import numpy as np

import concourse.bacc as bacc
import concourse.mybir as mybir
import concourse.tile as tile
from concourse.bass_utils import run_bass_kernel_spmd

F32 = mybir.dt.float32
BF16 = mybir.dt.bfloat16
FP8 = mybir.dt.float8e4
U16 = mybir.dt.uint16
AF = mybir.ActivationFunctionType
ALU = mybir.AluOpType
AXIS = mybir.AxisListType
PM = mybir.MatmulPerfMode

TEMPERATURE = 0.5
SCALE = float(math.exp(TEMPERATURE))
TSCALE = 16.0  # fp8 scale applied to normalized target rows

# Full problem config (hardcoded per contest rules).
B, L, C = 4, 2048, 512
N_CORES = 8
N_TOTAL = B * L                  # 8192
M_LOCAL = N_TOTAL // N_CORES     # 1024 rows per core

NB = 4                           # n blocks of 2048 columns (psum tiles)
BLK = N_TOTAL // NB              # 2048
QR = 1024                        # rows per octo load tile [128, 8, 512]
HALF = C // 2                    # 256: fp8 pair partner offset
MT = M_LOCAL // 128              # 8 output row tiles
JT = BLK // 512                  # 4 psum 512-col slices per block
OPB = BLK // QR                  # 2 octo loads per n-block


def octo_dram_ap(t2d, r0, rows=QR):
    # DRAM rows [r0, r0+rows) as [128 part, rows//128, c]; row = r0 + 8p + q
    return t2d[r0:r0 + rows, :].rearrange("(p q) c -> p q c", p=128)


def build_nc():
    m_local, n, c = M_LOCAL, N_TOTAL, C

    nc = bacc.Bacc("TRN2", target_bir_lowering=False, debug=False)
    pred = nc.dram_tensor("pred", [m_local, c], F32, kind="ExternalInput").ap()
    tgt = nc.dram_tensor("tgt", [n, c], F32, kind="ExternalInput").ap()
    td = nc.dram_tensor("td", [m_local, c], F32, kind="ExternalInput").ap()
    out = nc.dram_tensor("out", [128, MT], F32, kind="ExternalOutput").ap()

    with tile.TileContext(nc) as tc:
        with (
            tc.tile_pool(name="pq", bufs=1) as p_pool,        # pred + td octos
            tc.tile_pool(name="tq", bufs=5) as t_pool,        # target octos
            tc.tile_pool(name="pk", bufs=10) as pk_pool,      # packed fp8 chunks
            tc.tile_pool(name="pT", bufs=1) as pT_pool,       # transposed packs
            tc.tile_pool(name="st", bufs=1) as stats_pool,
            tc.tile_pool(name="sc", bufs=4) as scratch_pool,
            tc.tile_pool(name="esc", bufs=2) as esc_pool,
            tc.tile_pool(name="psum", bufs=2, space="PSUM") as psum_pool,
        ):
            # ---------- persistent stats ----------
            sp = stats_pool.tile([128, MT], F32)          # sum pred^2
            st = stats_pool.tile([128, MT], F32)          # sum td^2
            stt = stats_pool.tile([128, 64], F32)         # sum tgt^2 per chunk
            rp16 = stats_pool.tile([128, MT], F32)        # SCALE/TSCALE/||p||
            rp = stats_pool.tile([128, MT], F32)
            rtd = stats_pool.tile([128, MT], F32)
            rtt = stats_pool.tile([128, 64], F32)         # TSCALE/||t||
            d0 = stats_pool.tile([128, MT], F32)          # sum pred*td
            sume = stats_pool.tile([128, MT * NB], F32)   # exp row sums

            def rsqrt_cols(dst, src, k0, ncols, scale):
                """dst[:, k0:k0+ncols] = scale/sqrt(src[:, k0:k0+ncols]) via
                exp(-0.5 ln x); ln/exp share one ACT table set."""
                t2 = scratch_pool.tile([128, ncols], F32, name="bn2")
                nc.scalar.activation(t2[:], src[:, k0:k0 + ncols], AF.Ln)
                nc.scalar.activation(dst[:, k0:k0 + ncols], t2[:], AF.Exp,
                                     scale=-0.5)
                if scale != 1.0:
                    nc.vector.tensor_scalar_mul(
                        dst[:, k0:k0 + ncols], dst[:, k0:k0 + ncols], scale)

            def pack_pairs(dst_pk, src_ap, engine, rt_ap=None):
                """fp32 [128, 512] -> fp8 pair-packed [128, 512B]: byte 2k+i
                = x[:, 256i + k] (optionally * rt row scale). Iteration order
                (k outer, i inner) keeps the byte WRITES contiguous; the
                strided fp32 reads are cheap by comparison."""
                pkv = dst_pk[:].rearrange("p (k two) -> p k two", two=2)
                sv = src_ap.rearrange("p (two k) -> p k two", two=2)
                if rt_ap is None:
                    engine.tensor_copy(pkv, sv)
                else:
                    engine.tensor_scalar_mul(pkv, sv, rt_ap)

            # ---------- pred + td loads ----------
            pq = p_pool.tile([128, MT, c], F32)            # kept until tail
            nc.sync.dma_start(pq[:], octo_dram_ap(pred, 0))
            tdq = p_pool.tile([128, MT, c], F32)           # kept until tail
            nc.sync.dma_start(tdq[:], octo_dram_ap(td, 0))

            # pred stats -> rp, rp16
            for s in range(MT):
                sq = scratch_pool.tile([128, c], BF16, name="sqp")
                nc.vector.scalar_tensor_tensor(
                    sq[:], pq[:, s], 1.0, pq[:, s], ALU.mult, ALU.mult,
                    accum_out=sp[:, s:s + 1])
            rsqrt_cols(rp, sp, 0, MT, 1.0)
            nc.vector.tensor_scalar_mul(rp16[:], rp[:], SCALE / TSCALE)

            # pred raw fp32 -> bf16 -> xbar transpose (middle dim = c-chunk)
            # -> fp8 cast into the contiguous [128, 2, M] dual-fp8 weights
            # layout walrus requires (k-pair dim NOT byte-interleaved).
            pbf = p_pool.tile([128, MT, c], BF16, name="pbf")
            pTb = pT_pool.tile([128, 4, m_local], BF16, name="pTb")
            for s in range(MT):
                nc.vector.tensor_copy(pbf[:, s], pq[:, s])
                # out[p, e, a] = in[a, 128e+p]: e = c-chunk, a = m col
                nc.sync.dma_start_transpose(
                    pTb[:, :, s * 128:(s + 1) * 128], pbf[:, s])
            # ppT[:, h, i, m] = pred[m-col, 128h + 256i + k'] in fp8
            ppT = pT_pool.tile([128, 2, 2, m_local], FP8, name="ppT")
            for h in range(2):
                for i in range(2):
                    nc.vector.tensor_copy(ppT[:, h, i], pTb[:, h + 2 * i])

            # ---------- target pipeline ----------
            tT = [pT_pool.tile([128, 2, 2 * BLK], FP8, name=f"tT{g}")
                  for g in range(NB)]

            def emit_tgt_loads(g):
                tqs = []
                for o in range(OPB):
                    tq = t_pool.tile([128, 8, c], F32, name="tq")
                    nc.sync.dma_start(
                        tq[:], octo_dram_ap(tgt, (g * OPB + o) * QR))
                    tqs.append(tq)
                return tqs

            def emit_tgt_prep(g, tqs):
                dstu = tT[g][:].bitcast(U16)               # [128, 2, BLK]
                for o in range(OPB):
                    tq = tqs[o]
                    cg0 = (g * OPB + o) * 8
                    for q in range(8):
                        ci = cg0 + q
                        sq = scratch_pool.tile([128, c], BF16, name="sqt")
                        nc.vector.scalar_tensor_tensor(
                            sq[:], tq[:, q], 1.0, tq[:, q], ALU.mult, ALU.mult,
                            accum_out=stt[:, ci:ci + 1])
                    rsqrt_cols(rtt, stt, cg0, 8, TSCALE)
                    for q in range(8):
                        ci = cg0 + q
                        bl = o * 8 + q                     # chunk idx in block
                        pk = pk_pool.tile([128, 2 * HALF], FP8, name="tpk")
                        pack_pairs(pk, tq[:, q], nc.vector,
                                   rtt[:, ci:ci + 1])
                        nc.sync.dma_start_transpose(
                            dstu[:, :, bl * 128:(bl + 1) * 128],
                            pk[:].bitcast(U16))

            def emit_mm_block(g):
                tv = [tT[g][:, h].rearrange("p (n two) -> p two n", two=2)
                      for h in range(2)]
                for m in range(MT):
                    ps = psum_pool.tile([128, BLK], F32, name="ps")
                    for h in range(2):
                        lhs = ppT[:, h, :, m * 128:(m + 1) * 128]
                        for j in range(JT):
                            nc.tensor.matmul(
                                ps[:, j * 512:(j + 1) * 512],
                                lhs,
                                tv[h][:, :, j * 512:(j + 1) * 512],
                                start=(h == 0), stop=(h == 1),
                                perf_mode=PM.DoubleRow)
                    esc = esc_pool.tile([128, BLK], BF16, name="esc")
                    nc.scalar.activation(
                        esc[:], ps[:], AF.Exp, scale=rp16[:, m:m + 1],
                        accum_out=sume[:, m * NB + g:m * NB + g + 1])

            # one-block lookahead: target prep for g+1 overlaps matmuls of g;
            # loads for g+1 are issued on Sync before g's transposes so the
            # DMA stream keeps ahead of the compute pipeline.
            tq0 = emit_tgt_loads(0)
            tq1 = emit_tgt_loads(1)
            emit_tgt_prep(0, tq0)
            tq2 = emit_tgt_loads(2)
            emit_tgt_prep(1, tq1)
            emit_mm_block(0)
            tq3 = emit_tgt_loads(3)
            emit_tgt_prep(2, tq2)
            emit_mm_block(1)
            emit_tgt_prep(3, tq3)
            emit_mm_block(2)
            emit_mm_block(3)

            # ---------- tail: exact diag + lse - diag ----------
            for s in range(MT):
                sq = scratch_pool.tile([128, c], BF16, name="sqd")
                nc.vector.scalar_tensor_tensor(
                    sq[:], tdq[:, s], 1.0, tdq[:, s], ALU.mult, ALU.mult,
                    accum_out=st[:, s:s + 1])
                sq2 = scratch_pool.tile([128, c], BF16, name="sqd")
                nc.vector.scalar_tensor_tensor(
                    sq2[:], pq[:, s], 1.0, tdq[:, s], ALU.mult, ALU.mult,
                    accum_out=d0[:, s:s + 1])
            rsqrt_cols(rtd, st, 0, MT, 1.0)
            dtmp = scratch_pool.tile([128, MT], F32)
            nc.vector.tensor_mul(dtmp[:], d0[:], rp[:])
            diag = scratch_pool.tile([128, MT], F32)
            nc.vector.scalar_tensor_tensor(
                diag[:], dtmp[:], SCALE, rtd[:], ALU.mult, ALU.mult)

            rowsum = scratch_pool.tile([128, MT], F32)
            nc.vector.tensor_reduce(
                rowsum[:], sume[:].rearrange("p (m g) -> p m g", g=NB),
                axis=AXIS.X, op=ALU.add)
            lse = scratch_pool.tile([128, MT], F32)
            nc.scalar.activation(lse[:], rowsum[:], AF.Ln)
            losst = scratch_pool.tile([128, MT], F32)
            nc.vector.tensor_sub(losst[:], lse[:], diag[:])
            nc.sync.dma_start(out[:], losst[:])

    nc.compile()
    return nc


_NC_CACHE = {}


def _get_nc():
    key = (M_LOCAL, N_TOTAL, C)
    if key not in _NC_CACHE:
        _NC_CACHE[key] = build_nc()
    return _NC_CACHE[key]


def run_cores(pred2d, tgt2d, trace=False):
    """Run the SPMD program on cores 0..7; returns (partials [8,128,mt], res)."""
    nc = _get_nc()
    in_maps = []
    for ci in range(N_CORES):
        r0 = ci * M_LOCAL
        in_maps.append({
            "pred": np.ascontiguousarray(pred2d[r0:r0 + M_LOCAL]),
            "tgt": np.ascontiguousarray(tgt2d),
            "td": np.ascontiguousarray(tgt2d[r0:r0 + M_LOCAL]),
        })
    res = run_bass_kernel_spmd(nc, in_maps, list(range(N_CORES)), trace=trace)
    partials = np.stack([res.results[i]["out"] for i in range(N_CORES)])
    return partials, res


def kernel(pred, target):
    pred2d = np.asarray(pred, dtype=np.float32).reshape(-1, C)
    tgt2d = np.asarray(target, dtype=np.float32).reshape(-1, C)
    partials, _ = run_cores(pred2d, tgt2d)
    loss = partials.astype(np.float64).sum() / float(N_TOTAL)
    return np.float32(loss)


# revision 26
# speedup vs baseline: 2.7435x; 1.0514x over previous
"""Trainium2 Bass kernel for cosine-similarity contrastive loss (CosSimLoss).

reference:
    p = l2norm(pred).reshape(-1, C); t = l2norm(target).reshape(-1, C)
    logits = (p @ t.T) * e^0.5
    loss = mean(logsumexp(logits, axis=1) - diag(logits))

Strategy (8 NeuronCores, data parallel over the N = B*L = 8192 row dim).
Each core gets a 1024-row pred shard plus the full target and computes its
1024 x 8192 logits tile. Key points:

- fp8 (e4m3) DoubleRow matmuls: both operands cast to fp8 and packed as
  (c, c+256) byte pairs inside u16 words; one matmul consumes 256
  contraction rows (2 fp8 MACs/cell/cycle) - 2x bf16 TensorE throughput.
  Contraction pair h covers c in {128h+k', 128h+k'+256} for k' in [0,128).
- pred is NOT normalized before the matmul: its norm reciprocal rp, the
  e^0.5 temperature, and the fp8 x16 target scale all fold into the EXP
  activation's per-partition scale vector. Target rows are normalized (x16)
  during the fp32->fp8 cast (a per-column scale can't be applied after the
  matmul).
- transposes (contraction dim -> partitions) are SBUF->SBUF DMA xbar
  transposes of packed u16 tiles - no DRAM bounce, no TensorE transposes,
  so all 8 PSUM banks double-buffer the matmul->exp pipeline.
- loads use 8KB/partition descriptors ([128, 8, 512] "octo" tiles, row =
  r0 + 8p + q). Row order is scrambled across partitions, which is harmless:
  the loss is a mean over rows of (lse - diag) computed in one consistent
  scrambled order, and the lse sum over columns is order-invariant.
- row norms via one-pass DVE bn_stats (sum x^2 from even/odd mean+var).
- the diagonal is computed exactly in fp32 from the pred shard and the
  matching target rows (separate pre-sliced "td" input, SPMD-friendly),
  overlapped with the matmul phase. |cos|<=1 so exp never overflows.

Per-row (logsumexp - diag) partials return as [128, 8] per core; the host
sums and divides by N.
"""
import math

import numpy as np

import concourse.bacc as bacc
import concourse.mybir as mybir
import concourse.tile as tile
from concourse.bass_utils import run_bass_kernel_spmd

F32 = mybir.dt.float32
BF16 = mybir.dt.bfloat16
FP8 = mybir.dt.float8e4
U16 = mybir.dt.uint16
AF = mybir.ActivationFunctionType
ALU = mybir.AluOpType
AXIS = mybir.AxisListType
PM = mybir.MatmulPerfMode

TEMPERATURE = 0.5
SCALE = float(math.exp(TEMPERATURE))
TSCALE = 16.0  # fp8 scale applied to normalized target rows

# Full problem config (hardcoded per contest rules).
B, L, C = 4, 2048, 512
N_CORES = 8
N_TOTAL = B * L                  # 8192
M_LOCAL = N_TOTAL // N_CORES     # 1024 rows per core

NB = 4                           # n blocks of 2048 columns (psum tiles)
BLK = N_TOTAL // NB              # 2048
QR = 1024                        # rows per octo load tile [128, 8, 512]
HALF = C // 2                    # 256: fp8 pair partner offset
MT = M_LOCAL // 128              # 8 output row tiles
JT = BLK // 512                  # 4 psum 512-col slices per block
OPB = BLK // QR                  # 2 octo loads per n-block


def octo_dram_ap(t2d, r0, rows=QR):
    # DRAM rows [r0, r0+rows) as [128 part, rows//128, c]; row = r0 + 8p + q
    return t2d[r0:r0 + rows, :].rearrange("(p q) c -> p q c", p=128)


def build_nc():
    m_local, n, c = M_LOCAL, N_TOTAL, C

    nc = bacc.Bacc("TRN2", target_bir_lowering=False, debug=False)
    pred = nc.dram_tensor("pred", [m_local, c], F32, kind="ExternalInput").ap()
    tgt = nc.dram_tensor("tgt", [n, c], F32, kind="ExternalInput").ap()
    td = nc.dram_tensor("td", [m_local, c], F32, kind="ExternalInput").ap()
    out = nc.dram_tensor("out", [128, MT], F32, kind="ExternalOutput").ap()

    with tile.TileContext(nc) as tc:
        with (
            tc.tile_pool(name="pq", bufs=1) as p_pool,        # pred + td octos
            tc.tile_pool(name="tq", bufs=5) as t_pool,        # target octos
            tc.tile_pool(name="pk", bufs=10) as pk_pool,      # packed fp8 chunks
            tc.tile_pool(name="pT", bufs=1) as pT_pool,       # transposed packs
            tc.tile_pool(name="st", bufs=1) as stats_pool,
            tc.tile_pool(name="sc", bufs=4) as scratch_pool,
            tc.tile_pool(name="esc", bufs=2) as esc_pool,
            tc.tile_pool(name="psum", bufs=2, space="PSUM") as psum_pool,
        ):
            # ---------- persistent stats ----------
            sp = stats_pool.tile([128, MT], F32)          # sum pred^2
            st = stats_pool.tile([128, MT], F32)          # sum td^2
            stt = stats_pool.tile([128, 64], F32)         # sum tgt^2 per chunk
            rp16 = stats_pool.tile([128, MT], F32)        # SCALE/TSCALE/||p||
            rp = stats_pool.tile([128, MT], F32)
            rtd = stats_pool.tile([128, MT], F32)
            rtt = stats_pool.tile([128, 64], F32)         # TSCALE/||t||
            d0 = stats_pool.tile([128, MT], F32)          # sum pred*td
            sume = stats_pool.tile([128, MT * NB], F32)   # exp row sums

            def rsqrt_cols(dst, src, k0, ncols, scale):
                """dst[:, k0:k0+ncols] = scale/sqrt(src[...]) entirely on DVE
                (quake initial guess + 2 Newton steps, ~1e-6 rel) so ScalarE
                keeps a single ACT table set (Exp) and the target pipeline
                never waits on ScalarE."""
                s_ap = src[:, k0:k0 + ncols]
                d_ap = dst[:, k0:k0 + ncols]
                yi = scratch_pool.tile([128, ncols], mybir.dt.int32, name="bnq")
                nc.vector.tensor_scalar(
                    yi[:], s_ap.bitcast(mybir.dt.int32), 1, None,
                    ALU.arith_shift_right)
                nc.vector.tensor_scalar(
                    yi[:], yi[:], -1, 0x5f3759df, ALU.mult, ALU.add)
                y = yi[:].bitcast(F32)
                t1 = scratch_pool.tile([128, ncols], F32, name="bn1")
                t2 = scratch_pool.tile([128, ncols], F32, name="bn2")
                # Newton 1: y = y * (1.5 - 0.5 s y^2)
                nc.vector.tensor_mul(t1[:], y, s_ap)           # s*y
                nc.vector.scalar_tensor_tensor(
                    t2[:], y, -0.5, t1[:], ALU.mult, ALU.mult)  # -0.5 s y^2
                nc.vector.tensor_scalar_add(t2[:], t2[:], 1.5)
                nc.vector.tensor_mul(d_ap, y, t2[:])
                # Newton 2 (+ final scale folded in)
                nc.vector.tensor_mul(t1[:], d_ap, s_ap)
                nc.vector.scalar_tensor_tensor(
                    t2[:], d_ap, -0.5, t1[:], ALU.mult, ALU.mult)
                nc.vector.tensor_scalar_add(t2[:], t2[:], 1.5)
                nc.vector.scalar_tensor_tensor(
                    d_ap, d_ap, scale, t2[:], ALU.mult, ALU.mult)

            def pack_pairs(dst_pk, src_ap, engine, rt_ap=None):
                """fp32 [128, 512] -> fp8 pair-packed [128, 512B]: byte 2k+i
                = x[:, 256i + k] (optionally * rt row scale). Iteration order
                (k outer, i inner) keeps the byte WRITES contiguous; the
                strided fp32 reads are cheap by comparison."""
                pkv = dst_pk[:].rearrange("p (k two) -> p k two", two=2)
                sv = src_ap.rearrange("p (two k) -> p k two", two=2)
                if rt_ap is None:
                    engine.tensor_copy(pkv, sv)
                else:
                    engine.tensor_scalar_mul(pkv, sv, rt_ap)

            # ---------- pred + td loads (issued from the ScalarE hwdge
            # queue: Sync is reserved for the xbar transpose stream) ----------
            pq = p_pool.tile([128, MT, c], F32)            # kept until tail
            nc.scalar.dma_start(pq[:], octo_dram_ap(pred, 0))
            tdq = p_pool.tile([128, MT, c], F32)           # kept until tail
            nc.scalar.dma_start(tdq[:], octo_dram_ap(td, 0))

            # pred raw fp32 -> bf16 -> xbar transpose (middle dim = c-chunk)
            # -> fp8 cast into the contiguous [128, 2, M] dual-fp8 weights
            # layout walrus requires (k-pair dim NOT byte-interleaved).
            # This is the critical path to the first matmul, so it goes first.
            pbf = p_pool.tile([128, MT, c], BF16, name="pbf")
            pTb = pT_pool.tile([128, 4, m_local], BF16, name="pTb")
            for s in range(MT):
                nc.vector.tensor_copy(pbf[:, s], pq[:, s])
                # out[p, e, a] = in[a, 128e+p]: e = c-chunk, a = m col
                nc.sync.dma_start_transpose(
                    pTb[:, :, s * 128:(s + 1) * 128], pbf[:, s])
            # ppT[:, h, i, m] = pred[m-col, 128h + 256i + k'] in fp8
            ppT = pT_pool.tile([128, 2, 2, m_local], FP8, name="ppT")
            for h in range(2):
                for i in range(2):
                    nc.vector.tensor_copy(ppT[:, h, i], pTb[:, h + 2 * i])

            def emit_pred_stats():
                # rp/rp16 are only needed by the first EXP (well after the
                # first matmul), so this is emitted after block 0's prep.
                for s in range(MT):
                    sq = scratch_pool.tile([128, c], BF16, name="sqp")
                    nc.vector.scalar_tensor_tensor(
                        sq[:], pq[:, s], 1.0, pq[:, s], ALU.mult, ALU.mult,
                        accum_out=sp[:, s:s + 1])
                rsqrt_cols(rp, sp, 0, MT, 1.0)
                nc.vector.tensor_scalar_mul(rp16[:], rp[:], SCALE / TSCALE)

            # ---------- target pipeline ----------
            tT = [pT_pool.tile([128, 2, 2 * BLK], FP8, name=f"tT{g}")
                  for g in range(NB)]

            def emit_tgt_loads(g):
                tqs = []
                for o in range(OPB):
                    tq = t_pool.tile([128, 8, c], F32, name="tq")
                    nc.scalar.dma_start(
                        tq[:], octo_dram_ap(tgt, (g * OPB + o) * QR))
                    tqs.append(tq)
                return tqs

            def emit_tgt_prep(g, tqs):
                dstu = tT[g][:].bitcast(U16)               # [128, 2, BLK]
                for o in range(OPB):
                    tq = tqs[o]
                    cg0 = (g * OPB + o) * 8
                    for q in range(8):
                        ci = cg0 + q
                        sq = scratch_pool.tile([128, c], BF16, name="sqt")
                        nc.vector.scalar_tensor_tensor(
                            sq[:], tq[:, q], 1.0, tq[:, q], ALU.mult, ALU.mult,
                            accum_out=stt[:, ci:ci + 1])
                    rsqrt_cols(rtt, stt, cg0, 8, TSCALE)
                    for q in range(8):
                        ci = cg0 + q
                        bl = o * 8 + q                     # chunk idx in block
                        pk = pk_pool.tile([128, 2 * HALF], FP8, name="tpk")
                        pack_pairs(pk, tq[:, q], nc.vector,
                                   rtt[:, ci:ci + 1])
                        nc.sync.dma_start_transpose(
                            dstu[:, :, bl * 128:(bl + 1) * 128],
                            pk[:].bitcast(U16))

            def emit_mm_block(g):
                tv = [tT[g][:, h].rearrange("p (n two) -> p two n", two=2)
                      for h in range(2)]
                for m in range(MT):
                    ps = psum_pool.tile([128, BLK], F32, name="ps")
                    for h in range(2):
                        lhs = ppT[:, h, :, m * 128:(m + 1) * 128]
                        for j in range(JT):
                            nc.tensor.matmul(
                                ps[:, j * 512:(j + 1) * 512],
                                lhs,
                                tv[h][:, :, j * 512:(j + 1) * 512],
                                start=(h == 0), stop=(h == 1),
                                perf_mode=PM.DoubleRow)
                    esc = esc_pool.tile([128, BLK], BF16, name="esc")
                    nc.scalar.activation(
                        esc[:], ps[:], AF.Exp, scale=rp16[:, m:m + 1],
                        accum_out=sume[:, m * NB + g:m * NB + g + 1])

            # one-block lookahead: target prep for g+1 overlaps matmuls of g;
            # loads for g+1 are issued on Sync before g's transposes so the
            # DMA stream keeps ahead of the compute pipeline.
            tq0 = emit_tgt_loads(0)
            tq1 = emit_tgt_loads(1)
            emit_tgt_prep(0, tq0)
            emit_pred_stats()
            tq2 = emit_tgt_loads(2)
            emit_tgt_prep(1, tq1)
            emit_mm_block(0)
            tq3 = emit_tgt_loads(3)
            emit_tgt_prep(2, tq2)
            emit_mm_block(1)
            emit_tgt_prep(3, tq3)
            emit_mm_block(2)
            emit_mm_block(3)

            # ---------- tail: exact diag + lse - diag ----------
            for s in range(MT):
                sq = scratch_pool.tile([128, c], BF16, name="sqd")
                nc.vector.scalar_tensor_tensor(
                    sq[:], tdq[:, s], 1.0, tdq[:, s], ALU.mult, ALU.mult,
                    accum_out=st[:, s:s + 1])
                sq2 = scratch_pool.tile([128, c], BF16, name="sqd")
                nc.vector.scalar_tensor_tensor(
                    sq2[:], pq[:, s], 1.0, tdq[:, s], ALU.mult, ALU.mult,
                    accum_out=d0[:, s:s + 1])
            rsqrt_cols(rtd, st, 0, MT, 1.0)
            dtmp = scratch_pool.tile([128, MT], F32)
            nc.vector.tensor_mul(dtmp[:], d0[:], rp[:])
            diag = scratch_pool.tile([128, MT], F32)
            nc.vector.scalar_tensor_tensor(
                diag[:], dtmp[:], SCALE, rtd[:], ALU.mult, ALU.mult)

            rowsum = scratch_pool.tile([128, MT], F32)
            nc.vector.tensor_reduce(
                rowsum[:], sume[:].rearrange("p (m g) -> p m g", g=NB),
                axis=AXIS.X, op=ALU.add)
            lse = scratch_pool.tile([128, MT], F32)
            nc.scalar.activation(lse[:], rowsum[:], AF.Ln)
            losst = scratch_pool.tile([128, MT], F32)
            nc.vector.tensor_sub(losst[:], lse[:], diag[:])
            nc.scalar.dma_start(out[:], losst[:])

    nc.compile()
    return nc


_NC_CACHE = {}


def _get_nc():
    key = (M_LOCAL, N_TOTAL, C)
    if key not in _NC_CACHE:
        _NC_CACHE[key] = build_nc()
    return _NC_CACHE[key]


def run_cores(pred2d, tgt2d, trace=False):
    """Run the SPMD program on cores 0..7; returns (partials [8,128,mt], res)."""
    nc = _get_nc()
    in_maps = []
    for ci in range(N_CORES):
        r0 = ci * M_LOCAL
        in_maps.append({
            "pred": np.ascontiguousarray(pred2d[r0:r0 + M_LOCAL]),
            "tgt": np.ascontiguousarray(tgt2d),
            "td": np.ascontiguousarray(tgt2d[r0:r0 + M_LOCAL]),
        })
    res = run_bass_kernel_spmd(nc, in_maps, list(range(N_CORES)), trace=trace)
    partials = np.stack([res.results[i]["out"] for i in range(N_CORES)])
    return partials, res


def kernel(pred, target):
    pred2d = np.asarray(pred, dtype=np.float32).reshape(-1, C)
    tgt2d = np.asarray(target, dtype=np.float32).reshape(-1, C)
    partials, _ = run_cores(pred2d, tgt2d)
    loss = partials.astype(np.float64).sum() / float(N_TOTAL)
    return np.float32(loss)


# revision 32
# speedup vs baseline: 3.4244x; 1.2482x over previous
"""Trainium2 Bass kernel for cosine-similarity contrastive loss (CosSimLoss).

reference:
    p = l2norm(pred).reshape(-1, C); t = l2norm(target).reshape(-1, C)
    logits = (p @ t.T) * e^0.5
    loss = mean(logsumexp(logits, axis=1) - diag(logits))

Strategy (8 NeuronCores, data parallel over the N = B*L = 8192 row dim).
Each core gets a 1024-row pred shard plus the full target and computes its
1024 x 8192 logits tile. Key points:

- fp8 (e4m3) DoubleRow matmuls: both operands cast to fp8 and packed as
  (c, c+256) byte pairs inside u16 words; one matmul consumes 256
  contraction rows (2 fp8 MACs/cell/cycle) - 2x bf16 TensorE throughput.
  Contraction pair h covers c in {128h+k', 128h+k'+256} for k' in [0,128).
- pred is NOT normalized before the matmul: its norm reciprocal rp, the
  e^0.5 temperature, and the fp8 x16 target scale all fold into the EXP
  activation's per-partition scale vector. Target rows are normalized (x16)
  during the fp32->fp8 cast (a per-column scale can't be applied after the
  matmul).
- transposes (contraction dim -> partitions) are SBUF->SBUF DMA xbar
  transposes of packed u16 tiles - no DRAM bounce, no TensorE transposes,
  so all 8 PSUM banks double-buffer the matmul->exp pipeline.
- loads use 8KB/partition descriptors ([128, 8, 512] "octo" tiles, row =
  r0 + 8p + q). Row order is scrambled across partitions, which is harmless:
  the loss is a mean over rows of (lse - diag) computed in one consistent
  scrambled order, and the lse sum over columns is order-invariant.
- row norms via one-pass DVE bn_stats (sum x^2 from even/odd mean+var).
- the diagonal is computed exactly in fp32 from the pred shard and the
  matching target rows (separate pre-sliced "td" input, SPMD-friendly),
  overlapped with the matmul phase. |cos|<=1 so exp never overflows.

Per-row (logsumexp - diag) partials return as [128, 8] per core; the host
sums and divides by N.
"""
import math

import numpy as np

import concourse.bacc as bacc
import concourse.mybir as mybir
import concourse.tile as tile
from concourse.bass_utils import run_bass_kernel_spmd

F32 = mybir.dt.float32
BF16 = mybir.dt.bfloat16
FP8 = mybir.dt.float8e4
U16 = mybir.dt.uint16
AF = mybir.ActivationFunctionType
ALU = mybir.AluOpType
AXIS = mybir.AxisListType
PM = mybir.MatmulPerfMode

TEMPERATURE = 0.5
SCALE = float(math.exp(TEMPERATURE))
TSCALE = 16.0  # fp8 scale applied to normalized target rows

# Full problem config (hardcoded per contest rules).
B, L, C = 4, 2048, 512
N_CORES = 8
N_TOTAL = B * L                  # 8192
M_LOCAL = N_TOTAL // N_CORES     # 1024 rows per core

NB = 4                           # n blocks of 2048 columns (psum tiles)
BLK = N_TOTAL // NB              # 2048
QR = 1024                        # rows per octo load tile [128, 8, 512]
HALF = C // 2                    # 256: fp8 pair partner offset
MT = M_LOCAL // 128              # 8 output row tiles
JT = BLK // 512                  # 4 psum 512-col slices per block
OPB = BLK // QR                  # 2 octo loads per n-block


def octo_dram_ap(t2d, r0, rows=QR):
    # DRAM rows [r0, r0+rows) as [128 part, rows//128, c]; row = r0 + 8p + q
    return t2d[r0:r0 + rows, :].rearrange("(p q) c -> p q c", p=128)


def build_nc():
    m_local, n, c = M_LOCAL, N_TOTAL, C

    nc = bacc.Bacc("TRN2", target_bir_lowering=False, debug=False)
    pred = nc.dram_tensor("pred", [m_local, c], F32, kind="ExternalInput").ap()
    tgt = nc.dram_tensor("tgt", [n, c], F32, kind="ExternalInput").ap()
    td = nc.dram_tensor("td", [m_local, c], F32, kind="ExternalInput").ap()
    out = nc.dram_tensor("out", [128, MT], F32, kind="ExternalOutput").ap()

    with tile.TileContext(nc) as tc:
        with (
            tc.tile_pool(name="pq", bufs=1) as p_pool,        # pred + td octos
            tc.tile_pool(name="tq", bufs=4) as t_pool,        # target octos
            tc.tile_pool(name="pk", bufs=3) as pk_pool,      # packed fp8 chunks
            tc.tile_pool(name="pT", bufs=1) as pT_pool,       # transposed packs
            tc.tile_pool(name="st", bufs=1) as stats_pool,
            tc.tile_pool(name="sc", bufs=4) as scratch_pool,
            tc.tile_pool(name="esc", bufs=2) as esc_pool,
            tc.tile_pool(name="psum", bufs=2, space="PSUM") as psum_pool,
        ):
            # ---------- persistent stats ----------
            sp = stats_pool.tile([128, MT], F32)          # sum pred^2
            st = stats_pool.tile([128, MT], F32)          # sum td^2
            stt = stats_pool.tile([128, 64], F32)         # sum tgt^2 per chunk
            rp16 = stats_pool.tile([128, MT], F32)        # SCALE/TSCALE/||p||
            rp = stats_pool.tile([128, MT], F32)
            rtd = stats_pool.tile([128, MT], F32)
            rtt = stats_pool.tile([128, 64], F32)         # TSCALE/||t||
            d0 = stats_pool.tile([128, MT], F32)          # sum pred*td
            sume = stats_pool.tile([128, MT * NB], F32)   # exp row sums

            def rsqrt_cols(dst, src, k0, ncols, scale):
                """dst[:, k0:k0+ncols] = scale/sqrt(src[...]) entirely on DVE
                (quake initial guess + 2 Newton steps, ~1e-6 rel) so ScalarE
                keeps a single ACT table set (Exp) and the target pipeline
                never waits on ScalarE."""
                s_ap = src[:, k0:k0 + ncols]
                d_ap = dst[:, k0:k0 + ncols]
                yi = scratch_pool.tile([128, ncols], mybir.dt.int32, name="bnq")
                nc.vector.tensor_scalar(
                    yi[:], s_ap.bitcast(mybir.dt.int32), 1, None,
                    ALU.arith_shift_right)
                nc.vector.tensor_scalar(
                    yi[:], yi[:], -1, 0x5f3759df, ALU.mult, ALU.add)
                y = yi[:].bitcast(F32)
                t1 = scratch_pool.tile([128, ncols], F32, name="bn1")
                t2 = scratch_pool.tile([128, ncols], F32, name="bn2")
                # Newton 1: y = y * (1.5 - 0.5 s y^2)
                nc.vector.tensor_mul(t1[:], y, s_ap)           # s*y
                nc.vector.scalar_tensor_tensor(
                    t2[:], y, -0.5, t1[:], ALU.mult, ALU.mult)  # -0.5 s y^2
                nc.vector.tensor_scalar_add(t2[:], t2[:], 1.5)
                nc.vector.tensor_mul(d_ap, y, t2[:])
                # Newton 2 (+ final scale folded in)
                nc.vector.tensor_mul(t1[:], d_ap, s_ap)
                nc.vector.scalar_tensor_tensor(
                    t2[:], d_ap, -0.5, t1[:], ALU.mult, ALU.mult)
                nc.vector.tensor_scalar_add(t2[:], t2[:], 1.5)
                nc.vector.scalar_tensor_tensor(
                    d_ap, d_ap, scale, t2[:], ALU.mult, ALU.mult)

            def pack_chunk(pko, q, src_ap, rt_ap):
                """Pack one [128, 512] fp32 chunk (row-scaled by rt) into the
                h-major per-octo fp8 pair buffer: byte(h*2048 + q*256 +
                2k' + i) = x[:, 256i + 128h + k'] * rt. Writes are two
                contiguous 256B runs; reads stride within fp32."""
                dst = pko[:].rearrange(
                    "p (h q k two) -> p q h k two", h=2, q=8, two=2)[:, q]
                src = src_ap.rearrange(
                    "p (two h k) -> p h k two", two=2, h=2)
                nc.vector.tensor_scalar_mul(dst, src, rt_ap)

            # ---------- pred + td loads (issued from the ScalarE hwdge
            # queue: Sync is reserved for the xbar transpose stream) ----------
            pq = p_pool.tile([128, MT, c], F32)            # kept until tail
            nc.scalar.dma_start(pq[:], octo_dram_ap(pred, 0))
            tdq = p_pool.tile([128, MT, c], F32)           # kept until tail
            nc.scalar.dma_start(tdq[:], octo_dram_ap(td, 0))

            # pred raw fp32 -> bf16 (k-major staging) -> ONE xbar transpose
            # (out[p, e, a] = in[a, 128e+p], e = (c-chunk, s)) -> fp8 cast
            # into the contiguous [128, 2, M] dual-fp8 weights layout walrus
            # requires (k-pair dim NOT byte-interleaved). Critical path to
            # the first matmul, so it goes first.
            pbf = p_pool.tile([128, 4, MT, 128], BF16, name="pbf")
            pTb = pT_pool.tile([128, 4, m_local], BF16, name="pTb")
            for s in range(MT):
                nc.vector.tensor_copy(
                    pbf[:, :, s],
                    pq[:, s].rearrange("p (k cl) -> p k cl", cl=128))
            nc.sync.dma_start_transpose(
                pTb[:].rearrange("p k m -> p (k m)").rearrange(
                    "p (e a) -> p e a", a=128),
                pbf[:].rearrange("p k s cl -> p (k s cl)"))
            # ppT[:, h, i, m] = pred[m-col, 128h + 256i + k'] in fp8
            ppT = pT_pool.tile([128, 2, 2, m_local], FP8, name="ppT")
            for h in range(2):
                for i in range(2):
                    nc.vector.tensor_copy(ppT[:, h, i], pTb[:, h + 2 * i])

            def emit_pred_stats():
                # rp/rp16 are only needed by the first EXP (well after the
                # first matmul), so this is emitted after block 0's prep.
                for s in range(MT):
                    sq = scratch_pool.tile([128, c], BF16, name="sqp")
                    nc.vector.scalar_tensor_tensor(
                        sq[:], pq[:, s], 1.0, pq[:, s], ALU.mult, ALU.mult,
                        accum_out=sp[:, s:s + 1])
                rsqrt_cols(rp, sp, 0, MT, 1.0)
                nc.vector.tensor_scalar_mul(rp16[:], rp[:], SCALE / TSCALE)

            # ---------- target pipeline ----------
            # per-octo fp8 pair tiles: byte(h*2048 + q*256 + 2a + i) =
            # t^[n=(o,q,a), 128h + 256i + k'] * TSCALE (h-major so one xbar
            # transpose per octo lands the whole tile)
            tT = [[pT_pool.tile([128, 2, 8, 128, 2], FP8, name=f"tT{g}_{o}")
                   for o in range(OPB)] for g in range(NB)]

            def emit_tgt_loads(g):
                tqs = []
                for o in range(OPB):
                    tq = t_pool.tile([128, 8, c], F32, name="tq")
                    nc.scalar.dma_start(
                        tq[:], octo_dram_ap(tgt, (g * OPB + o) * QR))
                    tqs.append(tq)
                return tqs

            def emit_tgt_prep(g, tqs):
                for o in range(OPB):
                    tq = tqs[o]
                    cg0 = (g * OPB + o) * 8
                    for q in range(8):
                        ci = cg0 + q
                        sq = scratch_pool.tile([128, c], BF16, name="sqt")
                        nc.vector.scalar_tensor_tensor(
                            sq[:], tq[:, q], 1.0, tq[:, q], ALU.mult, ALU.mult,
                            accum_out=stt[:, ci:ci + 1])
                    rsqrt_cols(rtt, stt, cg0, 8, TSCALE)
                    pko = pk_pool.tile([128, 4096], FP8, name="tpk")
                    for q in range(8):
                        pack_chunk(pko, q, tq[:, q], rtt[:, cg0 + q:cg0 + q + 1])
                    # one xbar op transposes the whole packed octo
                    nc.sync.dma_start_transpose(
                        tT[g][o][:].rearrange(
                            "p h q a two -> p (h q) (a two)").bitcast(U16),
                        pko[:].bitcast(U16))

            def emit_mm_block(g):
                tv = [[tT[g][o][:, h].rearrange("p q a two -> p two (q a)")
                       for h in range(2)] for o in range(OPB)]
                for m in range(MT):
                    ps = psum_pool.tile([128, BLK], F32, name="ps")
                    for h in range(2):
                        lhs = ppT[:, h, :, m * 128:(m + 1) * 128]
                        for j in range(JT):
                            o, jl = divmod(j, 2)
                            nc.tensor.matmul(
                                ps[:, j * 512:(j + 1) * 512],
                                lhs,
                                tv[o][h][:, :, jl * 512:(jl + 1) * 512],
                                start=(h == 0), stop=(h == 1),
                                perf_mode=PM.DoubleRow)
                    esc = esc_pool.tile([128, BLK], BF16, name="esc")
                    nc.scalar.activation(
                        esc[:], ps[:], AF.Exp, scale=rp16[:, m:m + 1],
                        accum_out=sume[:, m * NB + g:m * NB + g + 1])

            # one-block lookahead: target prep for g+1 overlaps matmuls of g;
            # loads for g+1 are issued on Sync before g's transposes so the
            # DMA stream keeps ahead of the compute pipeline.
            tq0 = emit_tgt_loads(0)
            tq1 = emit_tgt_loads(1)
            emit_tgt_prep(0, tq0)
            emit_pred_stats()
            tq2 = emit_tgt_loads(2)
            emit_tgt_prep(1, tq1)
            emit_mm_block(0)
            tq3 = emit_tgt_loads(3)
            emit_tgt_prep(2, tq2)
            emit_mm_block(1)
            emit_tgt_prep(3, tq3)
            emit_mm_block(2)
            emit_mm_block(3)

            # ---------- tail: exact diag + lse - diag ----------
            for s in range(MT):
                sq = scratch_pool.tile([128, c], BF16, name="sqd")
                nc.vector.scalar_tensor_tensor(
                    sq[:], tdq[:, s], 1.0, tdq[:, s], ALU.mult, ALU.mult,
                    accum_out=st[:, s:s + 1])
                sq2 = scratch_pool.tile([128, c], BF16, name="sqd")
                nc.vector.scalar_tensor_tensor(
                    sq2[:], pq[:, s], 1.0, tdq[:, s], ALU.mult, ALU.mult,
                    accum_out=d0[:, s:s + 1])
            rsqrt_cols(rtd, st, 0, MT, 1.0)
            dtmp = scratch_pool.tile([128, MT], F32)
            nc.vector.tensor_mul(dtmp[:], d0[:], rp[:])
            diag = scratch_pool.tile([128, MT], F32)
            nc.vector.scalar_tensor_tensor(
                diag[:], dtmp[:], SCALE, rtd[:], ALU.mult, ALU.mult)

            rowsum = scratch_pool.tile([128, MT], F32)
            nc.vector.tensor_reduce(
                rowsum[:], sume[:].rearrange("p (m g) -> p m g", g=NB),
                axis=AXIS.X, op=ALU.add)
            lse = scratch_pool.tile([128, MT], F32)
            nc.scalar.activation(lse[:], rowsum[:], AF.Ln)
            losst = scratch_pool.tile([128, MT], F32)
            nc.vector.tensor_sub(losst[:], lse[:], diag[:])
            nc.scalar.dma_start(out[:], losst[:])

    nc.compile()
    return nc


_NC_CACHE = {}


def _get_nc():
    key = (M_LOCAL, N_TOTAL, C)
    if key not in _NC_CACHE:
        _NC_CACHE[key] = build_nc()
    return _NC_CACHE[key]


def run_cores(pred2d, tgt2d, trace=False):
    """Run the SPMD program on cores 0..7; returns (partials [8,128,mt], res)."""
    nc = _get_nc()
    in_maps = []
    for ci in range(N_CORES):
        r0 = ci * M_LOCAL
        in_maps.append({
            "pred": np.ascontiguousarray(pred2d[r0:r0 + M_LOCAL]),
            "tgt": np.ascontiguousarray(tgt2d),
            "td": np.ascontiguousarray(tgt2d[r0:r0 + M_LOCAL]),
        })
    res = run_bass_kernel_spmd(nc, in_maps, list(range(N_CORES)), trace=trace)
    partials = np.stack([res.results[i]["out"] for i in range(N_CORES)])
    return partials, res


def kernel(pred, target):
    pred2d = np.asarray(pred, dtype=np.float32).reshape(-1, C)
    tgt2d = np.asarray(target, dtype=np.float32).reshape(-1, C)
    partials, _ = run_cores(pred2d, tgt2d)
    loss = partials.astype(np.float64).sum() / float(N_TOTAL)
    return np.float32(loss)


# revision 37
# speedup vs baseline: 3.5422x; 1.0344x over previous
"""Trainium2 Bass kernel for cosine-similarity contrastive loss (CosSimLoss).

reference:
    p = l2norm(pred).reshape(-1, C); t = l2norm(target).reshape(-1, C)
    logits = (p @ t.T) * e^0.5
    loss = mean(logsumexp(logits, axis=1) - diag(logits))

Strategy (8 NeuronCores, data parallel over the N = B*L = 8192 row dim).
Each core gets a 1024-row pred shard plus the full target and computes its
1024 x 8192 logits tile. Key points:

- fp8 (e4m3) DoubleRow matmuls: both operands cast to fp8 and packed as
  (c, c+256) byte pairs inside u16 words; one matmul consumes 256
  contraction rows (2 fp8 MACs/cell/cycle) - 2x bf16 TensorE throughput.
  Contraction pair h covers c in {128h+k', 128h+k'+256} for k' in [0,128).
- pred is NOT normalized before the matmul: its norm reciprocal rp, the
  e^0.5 temperature, and the fp8 x16 target scale all fold into the EXP
  activation's per-partition scale vector. Target rows are normalized (x16)
  during the fp32->fp8 cast (a per-column scale can't be applied after the
  matmul).
- transposes (contraction dim -> partitions) are SBUF->SBUF DMA xbar
  transposes of packed u16 tiles - no DRAM bounce, no TensorE transposes,
  so all 8 PSUM banks double-buffer the matmul->exp pipeline.
- loads use 8KB/partition descriptors ([128, 8, 512] "octo" tiles, row =
  r0 + 8p + q). Row order is scrambled across partitions, which is harmless:
  the loss is a mean over rows of (lse - diag) computed in one consistent
  scrambled order, and the lse sum over columns is order-invariant.
- row norms via one-pass DVE bn_stats (sum x^2 from even/odd mean+var).
- the diagonal is computed exactly in fp32 from the pred shard and the
  matching target rows (separate pre-sliced "td" input, SPMD-friendly),
  overlapped with the matmul phase. |cos|<=1 so exp never overflows.

Per-row (logsumexp - diag) partials return as [128, 8] per core; the host
sums and divides by N.
"""
import math

import numpy as np

import concourse.bacc as bacc
import concourse.mybir as mybir
import concourse.tile as tile
from concourse.bass_utils import run_bass_kernel_spmd

F32 = mybir.dt.float32
BF16 = mybir.dt.bfloat16
FP8 = mybir.dt.float8e4
U16 = mybir.dt.uint16
AF = mybir.ActivationFunctionType
ALU = mybir.AluOpType
AXIS = mybir.AxisListType
PM = mybir.MatmulPerfMode

TEMPERATURE = 0.5
SCALE = float(math.exp(TEMPERATURE))
TSCALE = 16.0  # fp8 scale applied to normalized target rows

# Full problem config (hardcoded per contest rules).
B, L, C = 4, 2048, 512
N_CORES = 8
N_TOTAL = B * L                  # 8192
M_LOCAL = N_TOTAL // N_CORES     # 1024 rows per core

NB = 4                           # n blocks of 2048 columns (psum tiles)
BLK = N_TOTAL // NB              # 2048
QR = 1024                        # rows per octo load tile [128, 8, 512]
HALF = C // 2                    # 256: fp8 pair partner offset
MT = M_LOCAL // 128              # 8 output row tiles
JT = BLK // 512                  # 4 psum 512-col slices per block
OPB = BLK // QR                  # 2 octo loads per n-block


def octo_dram_ap(t2d, r0, rows=QR):
    # DRAM rows [r0, r0+rows) as [128 part, rows//128, c]; row = r0 + 8p + q
    return t2d[r0:r0 + rows, :].rearrange("(p q) c -> p q c", p=128)


def build_nc():
    m_local, n, c = M_LOCAL, N_TOTAL, C

    nc = bacc.Bacc("TRN2", target_bir_lowering=False, debug=False)
    pred = nc.dram_tensor("pred", [m_local, c], F32, kind="ExternalInput").ap()
    tgt = nc.dram_tensor("tgt", [n, c], F32, kind="ExternalInput").ap()
    td = nc.dram_tensor("td", [m_local, c], F32, kind="ExternalInput").ap()
    out = nc.dram_tensor("out", [128, MT], F32, kind="ExternalOutput").ap()

    with tile.TileContext(nc) as tc:
        with (
            tc.tile_pool(name="pq", bufs=1) as p_pool,        # pred + td octos
            tc.tile_pool(name="tq", bufs=4) as t_pool,        # target octos
            tc.tile_pool(name="pk", bufs=3) as pk_pool,      # packed fp8 chunks
            tc.tile_pool(name="pT", bufs=1) as pT_pool,       # transposed packs
            tc.tile_pool(name="st", bufs=1) as stats_pool,
            tc.tile_pool(name="sc", bufs=4) as scratch_pool,
            tc.tile_pool(name="esc", bufs=2) as esc_pool,
            tc.tile_pool(name="psum", bufs=2, space="PSUM") as psum_pool,
        ):
            # ---------- persistent stats ----------
            sp = stats_pool.tile([128, MT], F32)          # sum pred^2
            st = stats_pool.tile([128, MT], F32)          # sum td^2
            stt = stats_pool.tile([128, 64], F32)         # sum tgt^2 per chunk
            rp16 = stats_pool.tile([128, MT], F32)        # SCALE/TSCALE/||p||
            rp = stats_pool.tile([128, MT], F32)
            rtd = stats_pool.tile([128, MT], F32)
            rtt = stats_pool.tile([128, 64], F32)         # TSCALE/||t||
            d0 = stats_pool.tile([128, MT], F32)          # sum pred*td
            sume = stats_pool.tile([128, MT * NB], F32)   # exp row sums

            def rsqrt_cols(dst, src, k0, ncols, scale):
                """dst[:, k0:k0+ncols] = scale/sqrt(src[...]) entirely on DVE
                (quake initial guess + 2 Newton steps, ~1e-6 rel) so ScalarE
                keeps a single ACT table set (Exp) and the target pipeline
                never waits on ScalarE."""
                s_ap = src[:, k0:k0 + ncols]
                d_ap = dst[:, k0:k0 + ncols]
                yi = scratch_pool.tile([128, ncols], mybir.dt.int32, name="bnq")
                nc.vector.tensor_scalar(
                    yi[:], s_ap.bitcast(mybir.dt.int32), 1, None,
                    ALU.arith_shift_right)
                nc.vector.tensor_scalar(
                    yi[:], yi[:], -1, 0x5f3759df, ALU.mult, ALU.add)
                y = yi[:].bitcast(F32)
                t1 = scratch_pool.tile([128, ncols], F32, name="bn1")
                t2 = scratch_pool.tile([128, ncols], F32, name="bn2")
                # Newton 1: y = y * (1.5 - 0.5 s y^2)
                nc.vector.tensor_mul(t1[:], y, s_ap)           # s*y
                nc.vector.scalar_tensor_tensor(
                    t2[:], y, -0.5, t1[:], ALU.mult, ALU.mult)  # -0.5 s y^2
                nc.vector.tensor_scalar_add(t2[:], t2[:], 1.5)
                nc.vector.tensor_mul(d_ap, y, t2[:])
                # Newton 2 (+ final scale folded in)
                nc.vector.tensor_mul(t1[:], d_ap, s_ap)
                nc.vector.scalar_tensor_tensor(
                    t2[:], d_ap, -0.5, t1[:], ALU.mult, ALU.mult)
                nc.vector.tensor_scalar_add(t2[:], t2[:], 1.5)
                nc.vector.scalar_tensor_tensor(
                    d_ap, d_ap, scale, t2[:], ALU.mult, ALU.mult)

            def pack_chunk(pko, q, src_ap, rt_ap):
                """Pack one [128, 512] fp32 chunk (row-scaled by rt) into the
                h-major per-octo fp8 pair buffer: byte(h*2048 + q*256 +
                2k' + i) = x[:, 256i + 128h + k'] * rt. Writes are two
                contiguous 256B runs; reads stride within fp32."""
                dst = pko[:].rearrange(
                    "p (h q k two) -> p q h k two", h=2, q=8, two=2)[:, q]
                src = src_ap.rearrange(
                    "p (two h k) -> p h k two", two=2, h=2)
                nc.vector.tensor_scalar_mul(dst, src, rt_ap)

            # ---------- pred + td loads (issued from the ScalarE hwdge
            # queue: Sync is reserved for the xbar transpose stream) ----------
            pq = p_pool.tile([128, MT, c], F32)            # kept until tail
            nc.scalar.dma_start(pq[:], octo_dram_ap(pred, 0))
            tdq = p_pool.tile([128, MT, c], F32)           # kept until tail
            nc.scalar.dma_start(tdq[:], octo_dram_ap(td, 0))

            # pred raw fp32 -> bf16 (k-major staging) -> ONE xbar transpose
            # (out[p, e, a] = in[a, 128e+p], e = (c-chunk, s)) -> fp8 cast
            # into the contiguous [128, 2, M] dual-fp8 weights layout walrus
            # requires (k-pair dim NOT byte-interleaved). Critical path to
            # the first matmul, so it goes first.
            pbf = p_pool.tile([128, 4, MT, 128], BF16, name="pbf")
            pTb = pT_pool.tile([128, 4, m_local], BF16, name="pTb")
            for s in range(MT):
                nc.vector.tensor_copy(
                    pbf[:, :, s],
                    pq[:, s].rearrange("p (k cl) -> p k cl", cl=128))
            nc.sync.dma_start_transpose(
                pTb[:].rearrange("p k m -> p (k m)").rearrange(
                    "p (e a) -> p e a", a=128),
                pbf[:].rearrange("p k s cl -> p (k s cl)"))
            # ppT[:, h, i, m] = pred[m-col, 128h + 256i + k'] in fp8
            ppT = pT_pool.tile([128, 2, 2, m_local], FP8, name="ppT")

            def emit_ppt_casts():
                for h in range(2):
                    for i in range(2):
                        nc.vector.tensor_copy(ppT[:, h, i], pTb[:, h + 2 * i])

            def emit_pred_stats():
                # rp/rp16 are only needed by the first EXP (well after the
                # first matmul), so this is emitted after block 0's prep.
                for s in range(MT):
                    sq = scratch_pool.tile([128, c], BF16, name="sqp")
                    nc.vector.scalar_tensor_tensor(
                        sq[:], pq[:, s], 1.0, pq[:, s], ALU.mult, ALU.mult,
                        accum_out=sp[:, s:s + 1])
                rsqrt_cols(rp, sp, 0, MT, 1.0)
                nc.vector.tensor_scalar_mul(rp16[:], rp[:], SCALE / TSCALE)

            # ---------- target pipeline ----------
            # per-octo fp8 pair tiles: byte(h*2048 + q*256 + 2a + i) =
            # t^[n=(o,q,a), 128h + 256i + k'] * TSCALE (h-major so one xbar
            # transpose per octo lands the whole tile)
            tT = [[pT_pool.tile([128, 2, 8, 128, 2], FP8, name=f"tT{g}_{o}")
                   for o in range(OPB)] for g in range(NB)]

            def emit_tgt_loads(g):
                tqs = []
                for o in range(OPB):
                    tq = t_pool.tile([128, 8, c], F32, name="tq")
                    nc.scalar.dma_start(
                        tq[:], octo_dram_ap(tgt, (g * OPB + o) * QR))
                    tqs.append(tq)
                return tqs

            def emit_tgt_prep_octo(g, o, tq):
                cg0 = (g * OPB + o) * 8
                for q in range(8):
                    ci = cg0 + q
                    sq = scratch_pool.tile([128, c], BF16, name="sqt")
                    nc.vector.scalar_tensor_tensor(
                        sq[:], tq[:, q], 1.0, tq[:, q], ALU.mult, ALU.mult,
                        accum_out=stt[:, ci:ci + 1])
                rsqrt_cols(rtt, stt, cg0, 8, TSCALE)
                pko = pk_pool.tile([128, 4096], FP8, name="tpk")
                for q in range(8):
                    pack_chunk(pko, q, tq[:, q], rtt[:, cg0 + q:cg0 + q + 1])
                # one xbar op transposes the whole packed octo
                nc.sync.dma_start_transpose(
                    tT[g][o][:].rearrange(
                        "p h q a two -> p (h q) (a two)").bitcast(U16),
                    pko[:].bitcast(U16))

            def emit_tgt_prep(g, tqs):
                for o in range(OPB):
                    emit_tgt_prep_octo(g, o, tqs[o])

            def emit_mm_block(g):
                tv = [[tT[g][o][:, h].rearrange("p q a two -> p two (q a)")
                       for h in range(2)] for o in range(OPB)]
                for m in range(MT):
                    ps = psum_pool.tile([128, BLK], F32, name="ps")
                    for h in range(2):
                        lhs = ppT[:, h, :, m * 128:(m + 1) * 128]
                        for j in range(JT):
                            o, jl = divmod(j, 2)
                            nc.tensor.matmul(
                                ps[:, j * 512:(j + 1) * 512],
                                lhs,
                                tv[o][h][:, :, jl * 512:(jl + 1) * 512],
                                start=(h == 0), stop=(h == 1),
                                perf_mode=PM.DoubleRow)
                    esc = esc_pool.tile([128, BLK], BF16, name="esc")
                    nc.scalar.activation(
                        esc[:], ps[:], AF.Exp, scale=rp16[:, m:m + 1],
                        accum_out=sume[:, m * NB + g:m * NB + g + 1])

            # one-block lookahead: target prep for g+1 overlaps matmuls of g;
            # loads for g+1 are issued on Sync before g's transposes so the
            # DMA stream keeps ahead of the compute pipeline.
            tq0 = emit_tgt_loads(0)
            tq1 = emit_tgt_loads(1)
            emit_tgt_prep_octo(0, 0, tq0[0])
            emit_ppt_casts()
            emit_tgt_prep_octo(0, 1, tq0[1])
            emit_pred_stats()
            tq2 = emit_tgt_loads(2)
            emit_tgt_prep(1, tq1)
            emit_mm_block(0)
            tq3 = emit_tgt_loads(3)
            emit_tgt_prep(2, tq2)
            emit_mm_block(1)
            emit_tgt_prep(3, tq3)
            emit_mm_block(2)
            emit_mm_block(3)

            # ---------- tail: exact diag + lse - diag ----------
            for s in range(MT):
                sq = scratch_pool.tile([128, c], BF16, name="sqd")
                nc.vector.scalar_tensor_tensor(
                    sq[:], tdq[:, s], 1.0, tdq[:, s], ALU.mult, ALU.mult,
                    accum_out=st[:, s:s + 1])
                sq2 = scratch_pool.tile([128, c], BF16, name="sqd")
                nc.vector.scalar_tensor_tensor(
                    sq2[:], pq[:, s], 1.0, tdq[:, s], ALU.mult, ALU.mult,
                    accum_out=d0[:, s:s + 1])
            rsqrt_cols(rtd, st, 0, MT, 1.0)
            dtmp = scratch_pool.tile([128, MT], F32)
            nc.vector.tensor_mul(dtmp[:], d0[:], rp[:])
            diag = scratch_pool.tile([128, MT], F32)
            nc.vector.scalar_tensor_tensor(
                diag[:], dtmp[:], SCALE, rtd[:], ALU.mult, ALU.mult)

            rowsum = scratch_pool.tile([128, MT], F32)
            nc.vector.tensor_reduce(
                rowsum[:], sume[:].rearrange("p (m g) -> p m g", g=NB),
                axis=AXIS.X, op=ALU.add)
            lse = scratch_pool.tile([128, MT], F32)
            nc.scalar.activation(lse[:], rowsum[:], AF.Ln)
            losst = scratch_pool.tile([128, MT], F32)
            nc.vector.tensor_sub(losst[:], lse[:], diag[:])
            nc.scalar.dma_start(out[:], losst[:])

    nc.compile()
    return nc


_NC_CACHE = {}


def _get_nc():
    key = (M_LOCAL, N_TOTAL, C)
    if key not in _NC_CACHE:
        _NC_CACHE[key] = build_nc()
    return _NC_CACHE[key]


def run_cores(pred2d, tgt2d, trace=False):
    """Run the SPMD program on cores 0..7; returns (partials [8,128,mt], res)."""
    nc = _get_nc()
    in_maps = []
    for ci in range(N_CORES):
        r0 = ci * M_LOCAL
        in_maps.append({
            "pred": np.ascontiguousarray(pred2d[r0:r0 + M_LOCAL]),
            "tgt": np.ascontiguousarray(tgt2d),
            "td": np.ascontiguousarray(tgt2d[r0:r0 + M_LOCAL]),
        })
    res = run_bass_kernel_spmd(nc, in_maps, list(range(N_CORES)), trace=trace)
    partials = np.stack([res.results[i]["out"] for i in range(N_CORES)])
    return partials, res


def kernel(pred, target):
    pred2d = np.asarray(pred, dtype=np.float32).reshape(-1, C)
    tgt2d = np.asarray(target, dtype=np.float32).reshape(-1, C)
    partials, _ = run_cores(pred2d, tgt2d)
    loss = partials.astype(np.float64).sum() / float(N_TOTAL)
    return np.float32(loss)
